# revision 3
# baseline (speedup 1.0000x reference)
"""Causal multi-head self-attention on 8 Trainium2 NeuronCores.

Problem (hardcoded): x [2, 2048, 1024] f32, Wq/Wk/Wv/Wo [1024, 1024] f32,
H=16 heads, Dh=64, causal softmax(QK^T/8)V then output projection.

Sharding (Megatron-style): 2-way data parallel over batch x 4-way tensor
parallel over heads.  Core c handles batch c//4 and heads 4*(c%4)..+3 (a
256-wide slice of the hidden dim).  Wq/Wk/Wv sliced column-wise, Wo
row-wise; each core emits a partial [2048, 1024] output (bf16) which the
host sums per batch.

Device dataflow per core (single fused pipeline, ascending q-blocks):
  - x^T supplied host-side (d on partitions); per-(dc, qn) column-sliced
    DMAs so block 0's inputs land first
  - per q-block qn: Q/K/V projections for the *next* block are batched
    between this block's attention and epilogue, so TensorE never waits
    on DMA and ScalarE exp overlaps projection matmuls
  - scores computed transposed S^T[k, q], 2 heads row-packed in the PE
    array (Dh=64 contraction); straddle (diagonal) key-tiles compute only
    the valid column range [lo:512)
  - triangular causal mask added into PSUM via identity-matmul of a
    single [128,128] bf16 tile, only on the 128-wide diagonal sub-block
  - one exp() per PSUM tile on ScalarE (scale=1/8 folded in; straddles
    use a single strided sliced activation)
  - A.V col-tiled: 2 heads per array pass (64+64 stationary columns,
    independent rhs streams), accumulated over key tiles with sliced
    straddle writes
  - softmax denominators from a separate 4-way col-tiled ones-column
    matmul (partitions 0/32/64/96 of one PSUM bank, accumulated over kt)
  - normalize: K=1 ones-matmul broadcast + DVE reciprocal + tensor_mul
  - Wo row-parallel per 128-row output tile; Wo matmuls of block qn are
    spread into block qn+1's key-tile loop to fill exp-wait bubbles
"""

import os
import sys
from contextlib import ExitStack

import numpy as np

try:
    import concourse.bass as bass
except ImportError:  # pragma: no cover - path fallback for fresh dirs
    for p in ("/opt/trn_rl_repo", "/root/.axon_site/_ro/trn_rl_repo"):
        if os.path.isdir(p) and p not in sys.path:
            sys.path.insert(0, p)
    import concourse.bass as bass

import ml_dtypes
import concourse.bacc as bacc
import concourse.mybir as mybir
import concourse.tile as tile
from concourse.bass_utils import run_bass_kernel_spmd

F32 = mybir.dt.float32
F32R = mybir.dt.float32r
BF16 = mybir.dt.bfloat16

KCFG = os.environ.get("KCFG", "faster")
_DT = {
    "fast": dict(proj=BF16, qk=BF16, av=BF16, wo=F32R),
    "faster": dict(proj=BF16, qk=BF16, av=BF16, wo=BF16),
}[KCFG]
KRECIP = os.environ.get("KRECIP", "fast")

B, S, D = 2, 2048, 1024
H, DH = 16, 64
NCORES = 8
HPC = 4          # heads per core
JPC = HPC * DH   # 256 hidden dims per core
QB = 512         # query block
KB = 128         # key tile
NQ = S // QB     # 4
NK = S // KB     # 16
MASK_VAL = -1e7

_CACHE = {}
LAST_RESULTS = None


def _np_dt(dt):
    return ml_dtypes.bfloat16 if dt == BF16 else np.float32


def _build_nc():
    proj_dt, qk_dt, av_dt, wo_dt = _DT["proj"], _DT["qk"], _DT["av"], _DT["wo"]
    nc = bacc.Bacc()
    xT = nc.dram_tensor("xT", [D, S], proj_dt, kind="ExternalInput")
    wqT = nc.dram_tensor("wqT", [D, JPC], proj_dt, kind="ExternalInput")
    wkT = nc.dram_tensor("wkT", [D, JPC], proj_dt, kind="ExternalInput")
    wvT = nc.dram_tensor("wvT", [D, JPC], proj_dt, kind="ExternalInput")
    woT = nc.dram_tensor("woT", [JPC, D], wo_dt, kind="ExternalInput")
    tri = nc.dram_tensor("tri", [KB, KB], BF16, kind="ExternalInput")
    ident = nc.dram_tensor("ident", [KB, KB], BF16, kind="ExternalInput")
    onesr = nc.dram_tensor("onesr", [1, 64], F32R, kind="ExternalInput")
    y = nc.dram_tensor("y", [S, D], BF16, kind="ExternalOutput")

    with tile.TileContext(nc) as tc:
        with (
            tc.tile_pool(name="const", bufs=1) as constp,
            tc.tile_pool(name="act", bufs=1) as actp,
            tc.tile_pool(name="e", bufs=8) as ep,
            tc.tile_pool(name="ps", bufs=2, space="PSUM") as psp,
            tc.tile_pool(name="avp", bufs=2, space="PSUM") as avp,
            tc.tile_pool(name="dnp", bufs=2, space="PSUM") as dnp,
        ):
            ident_sb = constp.tile([KB, KB], BF16)
            tri_sb = constp.tile([KB, KB], BF16)
            ones_sb = constp.tile([1, 64], F32R)
            onescol = constp.tile([KB, 1], av_dt)
            wo_sb = actp.tile([128, 2, D], wo_dt)
            xT_sb = actp.tile([128, 8, S], proj_dt)
            wq_sb = actp.tile([128, 8, JPC], proj_dt)
            wk_sb = actp.tile([128, 8, JPC], proj_dt)
            wv_sb = actp.tile([128, 8, JPC], proj_dt)
            # QT/KT: [128, S] pair tiles; rows 0:64 head 2*pi, 64:128 2*pi+1
            QT = [actp.tile([128, S], qk_dt, name=f"QT{i}") for i in range(2)]
            KT = [actp.tile([128, S], qk_dt, name=f"KT{i}") for i in range(2)]
            V1 = actp.tile([128, NK, HPC, DH], av_dt)
            OT = [actp.tile([128, S], av_dt, name=f"OT{i}") for i in range(2)]
            sums_sb = actp.tile([1, HPC, S], F32R, name="sums_sb")

            # ---------------- DMA issue ----------------
            # sync: xT qn 0,1; scalar: xT qn 3; gpsimd: weights+consts+xT qn2
            for qn in (0, 1):
                for dc in range(8):
                    nc.sync.dma_start(
                        out=xT_sb[:, dc, qn * QB : (qn + 1) * QB],
                        in_=xT[dc * 128 : (dc + 1) * 128, qn * QB : (qn + 1) * QB],
                    )
            nc.gpsimd.dma_start(
                out=wq_sb[:], in_=wqT.rearrange("(c p) j -> p c j", p=128)
            )
            nc.gpsimd.dma_start(
                out=wk_sb[:], in_=wkT.rearrange("(c p) j -> p c j", p=128)
            )
            nc.gpsimd.dma_start(out=tri_sb[:], in_=tri[:])
            nc.gpsimd.dma_start(
                out=wv_sb[:], in_=wvT.rearrange("(c p) j -> p c j", p=128)
            )
            nc.gpsimd.dma_start(out=ident_sb[:], in_=ident[:])
            nc.gpsimd.dma_start(
                out=wo_sb[:], in_=woT.rearrange("(c p) j -> p c j", p=128)
            )
            nc.gpsimd.dma_start(out=ones_sb[:], in_=onesr[:])
            nc.gpsimd.memset(onescol[:], 1.0)
            for dc in range(8):
                nc.gpsimd.dma_start(
                    out=xT_sb[:, dc, 2 * QB : 3 * QB],
                    in_=xT[dc * 128 : (dc + 1) * 128, 2 * QB : 3 * QB],
                )
            for dc in range(8):
                nc.scalar.dma_start(
                    out=xT_sb[:, dc, 3 * QB : 4 * QB],
                    in_=xT[dc * 128 : (dc + 1) * 128, 3 * QB : 4 * QB],
                )

            # ---------------- helpers ----------------
            def qk_group(w_sb, out_tiles, mj, qn):
                ps = psp.tile([128, 1024], F32, tag="mm", name="ps_qk")
                for dc in range(8):
                    nc.tensor.matmul(
                        ps[:, :QB],
                        lhsT=w_sb[:, dc, mj * 128 : (mj + 1) * 128],
                        rhs=xT_sb[:, dc, qn * QB : (qn + 1) * QB],
                        start=(dc == 0),
                        stop=(dc == 7),
                    )
                nc.vector.tensor_copy(
                    out_tiles[mj][:, qn * QB : (qn + 1) * QB], ps[:, :QB]
                )

            def v_group(st):
                ps = psp.tile([128, 1024], F32, tag="mm", name="ps_v")
                for dc in range(8):
                    nc.tensor.matmul(
                        ps[:, :JPC],
                        lhsT=xT_sb[:, dc, st * 128 : (st + 1) * 128],
                        rhs=wv_sb[:, dc, :],
                        start=(dc == 0),
                        stop=(dc == 7),
                    )
                nc.vector.tensor_copy(
                    V1[:, st, :, :],
                    ps[:, :JPC].rearrange("p (h d) -> p h d", h=HPC),
                )

            def proj_block(qn):
                for st in range(4 * qn, 4 * qn + 4):
                    v_group(st)
                for mj in range(2):
                    qk_group(wq_sb, QT, mj, qn)
                for mj in range(2):
                    qk_group(wk_sb, KT, mj, qn)

            def emit_scores_exp(qn, kt):
                """Returns E tile pair for this key tile."""
                straddle = kt >= 4 * qn
                lo = 128 * (kt - 4 * qn) if straddle else 0
                E = []
                for pi in range(2):
                    ps = psp.tile([128, 1024], F32, tag="mm", name="ps_sc")
                    for hh in range(2):
                        nc.tensor.matmul(
                            ps[:, hh * QB + lo : (hh + 1) * QB],
                            lhsT=KT[pi][
                                hh * 64 : (hh + 1) * 64,
                                kt * KB : (kt + 1) * KB,
                            ],
                            rhs=QT[pi][
                                hh * 64 : (hh + 1) * 64,
                                qn * QB + lo : (qn + 1) * QB,
                            ],
                            start=True,
                            stop=not straddle,
                            tile_position=(hh * 64, 0),
                        )
                    if straddle:
                        for hh in range(2):
                            nc.tensor.matmul(
                                ps[:, hh * QB + lo : hh * QB + lo + 128],
                                lhsT=ident_sb,
                                rhs=tri_sb,
                                start=False,
                                stop=True,
                            )
                    e = ep.tile([128, 1024], av_dt, tag="e", name="e")
                    if straddle and lo > 0:
                        psv = ps[:].rearrange("p (h q) -> p h q", h=2)
                        ev = e[:].rearrange("p (h q) -> p h q", h=2)
                        nc.scalar.activation(
                            ev[:, :, lo:],
                            psv[:, :, lo:],
                            mybir.ActivationFunctionType.Exp,
                            scale=0.125,
                        )
                    else:
                        nc.scalar.activation(
                            e[:],
                            ps[:],
                            mybir.ActivationFunctionType.Exp,
                            scale=0.125,
                        )
                    E.append(e)
                return E

            def emit_av_dn(qn, kt, E, av, dn_ps, nkt):
                straddle = kt >= 4 * qn
                lo = 128 * (kt - 4 * qn) if straddle else 0
                first, last = kt == 0, kt == nkt - 1
                for p in range(2):
                    for j in range(2):
                        nc.tensor.matmul(
                            av[p][64 * j : 64 * (j + 1), lo:QB],
                            lhsT=V1[:, kt, 2 * p + j, :],
                            rhs=E[p][:, j * QB + lo : (j + 1) * QB],
                            start=first,
                            stop=last,
                            tile_position=(0, 64 * j),
                        )
                for h in range(HPC):
                    p, j = h // 2, h % 2
                    nc.tensor.matmul(
                        dn_ps[32 * h : 32 * h + 1, lo:QB],
                        lhsT=onescol[:],
                        rhs=E[p][:, j * QB + lo : (j + 1) * QB],
                        start=first,
                        stop=last,
                        tile_position=(0, 32 * h),
                    )

            def emit_wo(qn, st, last_block):
                ps_y = psp.tile([128, 1024], F32, tag="mm", name="ps_y")
                for nn in range(2):
                    for p in range(2):
                        nc.tensor.matmul(
                            ps_y[:, nn * QB : (nn + 1) * QB],
                            lhsT=OT[p][:, st * 128 : (st + 1) * 128],
                            rhs=wo_sb[:, p, nn * QB : (nn + 1) * QB],
                            start=(p == 0),
                            stop=(p == 1),
                        )
                y_sb = latep.tile([128, D], av_dt, tag="y", bufs=3, name="y_sb")
                if last_block:
                    nc.scalar.copy(y_sb[:], ps_y[:])
                else:
                    nc.vector.tensor_copy(y_sb[:], ps_y[:])
                oeng = nc.sync if st % 2 == 0 else nc.scalar
                oeng.dma_start(out=y[st * 128 : (st + 1) * 128, :], in_=y_sb[:])

            def epilogue_norm(qn, av, dn_ps):
                qs = slice(qn * QB, (qn + 1) * QB)
                for p in range(2):
                    nc.vector.tensor_copy(OT[p][:, qs], av[p][:])
                for h in range(HPC):
                    nc.vector.tensor_copy(
                        sums_sb[0:1, h, qs], dn_ps[32 * h : 32 * h + 1, :]
                    )
                for p in range(2):
                    rb = ep.tile([128, QB], F32, tag="rb", name="rb")
                    for j in range(2):
                        rb_ps = avp.tile([64, QB], F32, tag="av", name="rb_ps")
                        nc.tensor.matmul(
                            rb_ps[:],
                            lhsT=ones_sb[:],
                            rhs=sums_sb[0:1, 2 * p + j, qs],
                            start=True,
                            stop=True,
                        )
                        if KRECIP != "fast":
                            nc.vector.reciprocal(
                                rb[j * 64 : (j + 1) * 64, :], rb_ps[:]
                            )
                        elif j == 0:
                            nc.vector.reciprocal_approx_fast(
                                out=rb[0:64, :], in_=rb_ps[:]
                            )
                        else:
                            # approx_fast mis-writes at partition base 64:
                            # compute at base 0, then copy up
                            tmp = ep.tile([64, QB], F32, tag="rbt", name="tmp")
                            nc.vector.reciprocal_approx_fast(
                                out=tmp[:], in_=rb_ps[:]
                            )
                            nc.vector.tensor_copy(rb[64:128, :], tmp[:])
                    nc.vector.tensor_mul(OT[p][:, qs], OT[p][:, qs], rb[:])

            # ---------------- pipelined main loop ----------------
            late_ctx = ExitStack()
            latep = late_ctx.enter_context(tc.tile_pool(name="late", bufs=1))

            proj_block(0)
            pending_wo = []  # (qn, st) of deferred output projections
            for qn in range(NQ):
                nkt = 4 * qn + 4
                last_block = qn == NQ - 1
                av = [
                    avp.tile([128, QB], F32, tag="av", name=f"av{p}")
                    for p in range(2)
                ]
                dn_ps = dnp.tile([128, QB], F32, tag="dn", name="dn_ps")
                prevE = None
                for kt in range(nkt):
                    if pending_wo and kt in (2, 5, 8, 11):
                        emit_wo(*pending_wo.pop(0), last_block=False)
                    E = emit_scores_exp(qn, kt)
                    if prevE is not None:
                        emit_av_dn(qn, kt - 1, prevE, av, dn_ps, nkt)
                    prevE = E
                emit_av_dn(qn, nkt - 1, prevE, av, dn_ps, nkt)
                # leftover deferred Wo (short blocks have few kt slots)
                while pending_wo:
                    emit_wo(*pending_wo.pop(0), last_block=False)
                if not last_block:
                    proj_block(qn + 1)
                epilogue_norm(qn, av, dn_ps)
                for st in range(4 * qn, 4 * qn + 4):
                    if last_block:
                        emit_wo(qn, st, last_block=True)
                    else:
                        pending_wo.append((qn, st))
            while pending_wo:
                emit_wo(*pending_wo.pop(0), last_block=True)
            late_ctx.close()
    return nc


def _get_nc():
    if "nc" not in _CACHE:
        nc = _build_nc()
        nc.finalize()
        _CACHE["nc"] = nc
    return _CACHE["nc"]


def _host_consts():
    rk = np.arange(KB)[:, None]
    rq = np.arange(KB)[None, :]
    tri = np.where(rq >= rk, 0.0, MASK_VAL).astype(ml_dtypes.bfloat16)
    identity = np.eye(KB, dtype=ml_dtypes.bfloat16)
    return tri, identity


def kernel(x, Wq, Wk, Wv, Wo):
    global LAST_RESULTS
    x = np.asarray(x, np.float32)
    Wq = np.asarray(Wq, np.float32)
    Wk = np.asarray(Wk, np.float32)
    Wv = np.asarray(Wv, np.float32)
    Wo = np.asarray(Wo, np.float32)

    pdt, wdt = _np_dt(_DT["proj"]), _np_dt(_DT["wo"])
    tri, identity = _host_consts()
    onesr_np = np.ones((1, 64), np.float32)
    xTs = [np.ascontiguousarray(x[b].T).astype(pdt) for b in range(B)]

    in_maps = []
    for c in range(NCORES):
        b, g = c // (NCORES // B), c % (NCORES // B)
        jsel = slice(g * JPC, (g + 1) * JPC)
        in_maps.append(
            {
                "xT": xTs[b],
                "wqT": np.ascontiguousarray(Wq[jsel].T).astype(pdt),
                "wkT": np.ascontiguousarray(Wk[jsel].T).astype(pdt),
                "wvT": np.ascontiguousarray(Wv[jsel].T).astype(pdt),
                "woT": np.ascontiguousarray(Wo[:, jsel].T).astype(wdt),
                "tri": tri,
                "ident": identity,
                "onesr": onesr_np,
            }
        )

    res = run_bass_kernel_spmd(_get_nc(), in_maps, list(range(NCORES)))
    LAST_RESULTS = res
    ys = [res.results[c]["y"].astype(np.float32) for c in range(NCORES)]
    npc = NCORES // B
    out = np.stack(
        [sum(ys[b * npc + 1 : (b + 1) * npc], ys[b * npc]) for b in range(B)]
    )
    return out.astype(np.float32)


# revision 11
# speedup vs baseline: 1.2620x; 1.2620x over previous
"""Causal multi-head self-attention on 8 Trainium2 NeuronCores.

Problem (hardcoded): x [2, 2048, 1024] f32, Wq/Wk/Wv/Wo [1024, 1024] f32,
H=16 heads, Dh=64, causal softmax(QK^T/8)V then output projection.

Sharding (Megatron-style): 2-way data parallel over batch x 4-way tensor
parallel over heads.  Core c handles batch c//4 and heads 4*(c%4)..+3 (a
256-wide slice of the hidden dim).  Wq/Wk/Wv sliced column-wise, Wo
row-wise; each core emits a partial [2048, 1024] output (bf16) which the
host sums per batch.

Device dataflow per core (single fused pipeline, ascending q-blocks):
  - x^T supplied host-side (d on partitions); per-(dc, qn) column-sliced
    DMAs so block 0's inputs land first
  - per q-block qn: Q/K/V projections for the *next* block are batched
    between this block's attention and epilogue, so TensorE never waits
    on DMA and ScalarE exp overlaps projection matmuls
  - scores computed transposed S^T[k, q], 2 heads row-packed in the PE
    array (Dh=64 contraction); straddle (diagonal) key-tiles compute only
    the valid column range [lo:512)
  - triangular causal mask added into PSUM via identity-matmul of a
    single [128,128] bf16 tile, only on the 128-wide diagonal sub-block
  - one exp() per PSUM tile on ScalarE (scale=1/8 folded in; straddles
    use a single strided sliced activation)
  - A.V with stationary [V | ones] so the softmax denominator appears as
    row 64 of the same matmul output; straddle key-tiles write only the
    valid column range (start flag on kt=0 which is always full-width)
  - normalize: K=1 ones-matmul broadcast + DVE reciprocal + tensor_mul
  - Wo row-parallel per 128-row output tile; Wo matmuls of block qn are
    spread into block qn+1's key-tile loop to fill exp-wait bubbles
"""

import os
import sys
from contextlib import ExitStack

import numpy as np

try:
    import concourse.bass as bass
except ImportError:  # pragma: no cover - path fallback for fresh dirs
    for p in ("/opt/trn_rl_repo", "/root/.axon_site/_ro/trn_rl_repo"):
        if os.path.isdir(p) and p not in sys.path:
            sys.path.insert(0, p)
    import concourse.bass as bass

import ml_dtypes
import concourse.bacc as bacc
import concourse.mybir as mybir
import concourse.tile as tile
from concourse.bass_utils import run_bass_kernel_spmd

F32 = mybir.dt.float32
F32R = mybir.dt.float32r
BF16 = mybir.dt.bfloat16

KCFG = os.environ.get("KCFG", "faster")
_DT = {
    "fast": dict(proj=BF16, qk=BF16, av=BF16, wo=F32R),
    "faster": dict(proj=BF16, qk=BF16, av=BF16, wo=BF16),
}[KCFG]
KRECIP = os.environ.get("KRECIP", "fast")

B, S, D = 2, 2048, 1024
H, DH = 16, 64
NCORES = 8
HPC = 4          # heads per core
JPC = HPC * DH   # 256 hidden dims per core
QB = 512         # query block
KB = 128         # key tile
NQ = S // QB     # 4
NK = S // KB     # 16
MASK_VAL = -1e7

_CACHE = {}
LAST_RESULTS = None


def _np_dt(dt):
    return ml_dtypes.bfloat16 if dt == BF16 else np.float32


def _build_nc():
    proj_dt, qk_dt, av_dt, wo_dt = _DT["proj"], _DT["qk"], _DT["av"], _DT["wo"]
    nc = bacc.Bacc()
    xT = nc.dram_tensor("xT", [D, S], proj_dt, kind="ExternalInput")
    wqT = nc.dram_tensor("wqT", [D, JPC], proj_dt, kind="ExternalInput")
    wkT = nc.dram_tensor("wkT", [D, JPC], proj_dt, kind="ExternalInput")
    wvT = nc.dram_tensor("wvT", [D, JPC], proj_dt, kind="ExternalInput")
    woT = nc.dram_tensor("woT", [JPC, D], wo_dt, kind="ExternalInput")
    tri = nc.dram_tensor("tri", [KB, KB], BF16, kind="ExternalInput")
    ident = nc.dram_tensor("ident", [KB, KB], BF16, kind="ExternalInput")
    onesr = nc.dram_tensor("onesr", [1, 64], F32R, kind="ExternalInput")
    y = nc.dram_tensor("y", [S, D], BF16, kind="ExternalOutput")

    with tile.TileContext(nc) as tc:
        with (
            tc.tile_pool(name="const", bufs=1) as constp,
            tc.tile_pool(name="act", bufs=1) as actp,
            tc.tile_pool(name="e", bufs=8) as ep,
            tc.tile_pool(name="ps", bufs=2, space="PSUM") as psp,
            tc.tile_pool(name="avp", bufs=4, space="PSUM") as avp,
        ):
            ident_sb = constp.tile([KB, KB], BF16)
            tri_sb = constp.tile([KB, KB], BF16)
            ones_sb = constp.tile([1, 64], F32R)
            wo_sb = actp.tile([128, 2, D], wo_dt)
            xT_sb = actp.tile([128, 8, S], proj_dt)
            wq_sb = actp.tile([128, 8, JPC], proj_dt)
            wk_sb = actp.tile([128, 8, JPC], proj_dt)
            wv_sb = actp.tile([128, 8, JPC], proj_dt)
            # QT/KT: [128, S] pair tiles; rows 0:64 head 2*pi, 64:128 2*pi+1
            QT = [actp.tile([128, S], qk_dt, name=f"QT{i}") for i in range(2)]
            KT = [actp.tile([128, S], qk_dt, name=f"KT{i}") for i in range(2)]
            # V with ones column appended per (k-tile, head): the softmax
            # denominator falls out of the A.V matmul as row 64
            V1 = actp.tile([128, NK, HPC, DH + 1], av_dt)
            OT = [actp.tile([128, S], av_dt, name=f"OT{i}") for i in range(2)]
            sums_sb = actp.tile([1, HPC, S], F32R, name="sums_sb")

            # ---------------- DMA issue ----------------
            # Per-queue DMA bandwidth is ~110GB/s: spread the 6MB of input
            # across sync/scalar/gpsimd/vector so block 0's operands (wq,
            # wk, wv, xT qn0, tri) land in parallel.
            nc.sync.dma_start(
                out=wq_sb[:], in_=wqT.rearrange("(c p) j -> p c j", p=128)
            )
            nc.scalar.dma_start(
                out=wk_sb[:], in_=wkT.rearrange("(c p) j -> p c j", p=128)
            )
            nc.gpsimd.dma_start(
                out=wv_sb[:], in_=wvT.rearrange("(c p) j -> p c j", p=128)
            )
            nc.gpsimd.dma_start(out=tri_sb[:], in_=tri[:])
            nc.gpsimd.dma_start(out=ident_sb[:], in_=ident[:])
            nc.gpsimd.dma_start(out=ones_sb[:], in_=onesr[:])
            nc.gpsimd.memset(V1[:, :, :, DH : DH + 1], 1.0)
            for qn in (0, 1):
                for dc in range(8):
                    eng = nc.sync if dc % 2 == 0 else nc.scalar
                    eng.dma_start(
                        out=xT_sb[:, dc, qn * QB : (qn + 1) * QB],
                        in_=xT[dc * 128 : (dc + 1) * 128, qn * QB : (qn + 1) * QB],
                    )
            nc.gpsimd.dma_start(
                out=wo_sb[:], in_=woT.rearrange("(c p) j -> p c j", p=128)
            )
            for dc in range(8):
                nc.gpsimd.dma_start(
                    out=xT_sb[:, dc, 2 * QB : 3 * QB],
                    in_=xT[dc * 128 : (dc + 1) * 128, 2 * QB : 3 * QB],
                )
            for dc in range(8):
                nc.sync.dma_start(
                    out=xT_sb[:, dc, 3 * QB : 4 * QB],
                    in_=xT[dc * 128 : (dc + 1) * 128, 3 * QB : 4 * QB],
                )

            # ---------------- helpers ----------------
            def qk_group(w_sb, out_tiles, mj, qn):
                ps = psp.tile([128, 1024], F32, tag="mm", name="ps_qk")
                for dc in range(8):
                    nc.tensor.matmul(
                        ps[:, :QB],
                        lhsT=w_sb[:, dc, mj * 128 : (mj + 1) * 128],
                        rhs=xT_sb[:, dc, qn * QB : (qn + 1) * QB],
                        start=(dc == 0),
                        stop=(dc == 7),
                    )
                nc.vector.tensor_copy(
                    out_tiles[mj][:, qn * QB : (qn + 1) * QB], ps[:, :QB]
                )

            def v_group(st):
                ps = psp.tile([128, 1024], F32, tag="mm", name="ps_v")
                for dc in range(8):
                    nc.tensor.matmul(
                        ps[:, :JPC],
                        lhsT=xT_sb[:, dc, st * 128 : (st + 1) * 128],
                        rhs=wv_sb[:, dc, :],
                        start=(dc == 0),
                        stop=(dc == 7),
                    )
                nc.vector.tensor_copy(
                    V1[:, st, :, 0:DH],
                    ps[:, :JPC].rearrange("p (h d) -> p h d", h=HPC),
                )

            def proj_block(qn):
                for mj in range(2):
                    qk_group(wq_sb, QT, mj, qn)
                for mj in range(2):
                    qk_group(wk_sb, KT, mj, qn)
                for st in range(4 * qn, 4 * qn + 4):
                    v_group(st)

            def emit_scores_exp(qn, kt):
                """Returns E tile pair for this key tile."""
                straddle = kt >= 4 * qn
                lo = 128 * (kt - 4 * qn) if straddle else 0
                E = []
                for pi in range(2):
                    ps = psp.tile([128, 1024], F32, tag="mm", name="ps_sc")
                    for hh in range(2):
                        nc.tensor.matmul(
                            ps[:, hh * QB + lo : (hh + 1) * QB],
                            lhsT=KT[pi][
                                hh * 64 : (hh + 1) * 64,
                                kt * KB : (kt + 1) * KB,
                            ],
                            rhs=QT[pi][
                                hh * 64 : (hh + 1) * 64,
                                qn * QB + lo : (qn + 1) * QB,
                            ],
                            start=True,
                            stop=not straddle,
                            tile_position=(hh * 64, 0),
                        )
                    if straddle:
                        for hh in range(2):
                            nc.tensor.matmul(
                                ps[:, hh * QB + lo : hh * QB + lo + 128],
                                lhsT=ident_sb,
                                rhs=tri_sb,
                                start=False,
                                stop=True,
                            )
                    e = ep.tile([128, 1024], av_dt, tag="e", name="e")
                    if straddle and lo > 0:
                        psv = ps[:].rearrange("p (h q) -> p h q", h=2)
                        ev = e[:].rearrange("p (h q) -> p h q", h=2)
                        nc.scalar.activation(
                            ev[:, :, lo:],
                            psv[:, :, lo:],
                            mybir.ActivationFunctionType.Exp,
                            scale=0.125,
                        )
                    else:
                        nc.scalar.activation(
                            e[:],
                            ps[:],
                            mybir.ActivationFunctionType.Exp,
                            scale=0.125,
                        )
                    E.append(e)
                return E

            def emit_av(qn, kt, E, av, nkt):
                straddle = kt >= 4 * qn
                lo = 128 * (kt - 4 * qn) if straddle else 0
                first, last = kt == 0, kt == nkt - 1
                for h in range(HPC):
                    p, j = h // 2, h % 2
                    nc.tensor.matmul(
                        av[h][:, lo:QB],
                        lhsT=V1[:, kt, h, :],
                        rhs=E[p][:, j * QB + lo : (j + 1) * QB],
                        start=first,
                        stop=last,
                    )

            def emit_wo(qn, st, last_block):
                ps_y = psp.tile([128, 1024], F32, tag="mm", name="ps_y")
                for nn in range(2):
                    for p in range(2):
                        nc.tensor.matmul(
                            ps_y[:, nn * QB : (nn + 1) * QB],
                            lhsT=OT[p][:, st * 128 : (st + 1) * 128],
                            rhs=wo_sb[:, p, nn * QB : (nn + 1) * QB],
                            start=(p == 0),
                            stop=(p == 1),
                        )
                y_sb = latep.tile([128, D], av_dt, tag="y", bufs=3, name="y_sb")
                if last_block:
                    nc.scalar.copy(y_sb[:], ps_y[:])
                else:
                    nc.vector.tensor_copy(y_sb[:], ps_y[:])
                oeng = nc.sync if st % 2 == 0 else nc.scalar
                oeng.dma_start(out=y[st * 128 : (st + 1) * 128, :], in_=y_sb[:])

            def epilogue_norm(qn, av):
                qs = slice(qn * QB, (qn + 1) * QB)
                for h in range(HPC):
                    p, j = h // 2, h % 2
                    nc.vector.tensor_copy(
                        OT[p][j * 64 : (j + 1) * 64, qs], av[h][0:DH, :]
                    )
                    nc.vector.tensor_copy(
                        sums_sb[0:1, h, qs], av[h][DH : DH + 1, :]
                    )
                for p in range(2):
                    rb = ep.tile([128, QB], F32, tag="rb", name="rb")
                    for j in range(2):
                        rb_ps = avp.tile([64, QB], F32, tag="av", name="rb_ps")
                        nc.tensor.matmul(
                            rb_ps[:],
                            lhsT=ones_sb[:],
                            rhs=sums_sb[0:1, 2 * p + j, qs],
                            start=True,
                            stop=True,
                        )
                        if KRECIP != "fast":
                            nc.vector.reciprocal(
                                rb[j * 64 : (j + 1) * 64, :], rb_ps[:]
                            )
                        elif j == 0:
                            nc.vector.reciprocal_approx_fast(
                                out=rb[0:64, :], in_=rb_ps[:]
                            )
                        else:
                            # approx_fast mis-writes at partition base 64:
                            # compute at base 0, then copy up
                            tmp = ep.tile([64, QB], F32, tag="rbt", name="tmp")
                            nc.vector.reciprocal_approx_fast(
                                out=tmp[:], in_=rb_ps[:]
                            )
                            nc.vector.tensor_copy(rb[64:128, :], tmp[:])
                    nc.vector.tensor_mul(OT[p][:, qs], OT[p][:, qs], rb[:])

            # ---------------- pipelined main loop ----------------
            late_ctx = ExitStack()
            latep = late_ctx.enter_context(tc.tile_pool(name="late", bufs=1))

            proj_block(0)
            pending_wo = []  # (qn, st) of deferred output projections
            for qn in range(NQ):
                nkt = 4 * qn + 4
                last_block = qn == NQ - 1
                av = [
                    avp.tile([DH + 1, QB], F32, tag="av", name=f"av{h}")
                    for h in range(HPC)
                ]
                prevE = None
                for kt in range(nkt):
                    if pending_wo and kt in (2, 5, 8, 11):
                        emit_wo(*pending_wo.pop(0), last_block=False)
                    E = emit_scores_exp(qn, kt)
                    if prevE is not None:
                        emit_av(qn, kt - 1, prevE, av, nkt)
                    prevE = E
                emit_av(qn, nkt - 1, prevE, av, nkt)
                # leftover deferred Wo (short blocks have few kt slots)
                while pending_wo:
                    emit_wo(*pending_wo.pop(0), last_block=False)
                if not last_block:
                    proj_block(qn + 1)
                epilogue_norm(qn, av)
                for st in range(4 * qn, 4 * qn + 4):
                    if last_block:
                        emit_wo(qn, st, last_block=True)
                    else:
                        pending_wo.append((qn, st))
            while pending_wo:
                emit_wo(*pending_wo.pop(0), last_block=True)
            late_ctx.close()
    return nc


def _get_nc():
    if "nc" not in _CACHE:
        nc = _build_nc()
        nc.finalize()
        _CACHE["nc"] = nc
    return _CACHE["nc"]


def _host_consts():
    rk = np.arange(KB)[:, None]
    rq = np.arange(KB)[None, :]
    tri = np.where(rq >= rk, 0.0, MASK_VAL).astype(ml_dtypes.bfloat16)
    identity = np.eye(KB, dtype=ml_dtypes.bfloat16)
    return tri, identity


def kernel(x, Wq, Wk, Wv, Wo):
    global LAST_RESULTS
    x = np.asarray(x, np.float32)
    Wq = np.asarray(Wq, np.float32)
    Wk = np.asarray(Wk, np.float32)
    Wv = np.asarray(Wv, np.float32)
    Wo = np.asarray(Wo, np.float32)

    pdt, wdt = _np_dt(_DT["proj"]), _np_dt(_DT["wo"])
    tri, identity = _host_consts()
    onesr_np = np.ones((1, 64), np.float32)
    xTs = [np.ascontiguousarray(x[b].T).astype(pdt) for b in range(B)]

    in_maps = []
    for c in range(NCORES):
        b, g = c // (NCORES // B), c % (NCORES // B)
        jsel = slice(g * JPC, (g + 1) * JPC)
        in_maps.append(
            {
                "xT": xTs[b],
                "wqT": np.ascontiguousarray(Wq[jsel].T).astype(pdt),
                "wkT": np.ascontiguousarray(Wk[jsel].T).astype(pdt),
                "wvT": np.ascontiguousarray(Wv[jsel].T).astype(pdt),
                "woT": np.ascontiguousarray(Wo[:, jsel].T).astype(wdt),
                "tri": tri,
                "ident": identity,
                "onesr": onesr_np,
            }
        )

    res = run_bass_kernel_spmd(_get_nc(), in_maps, list(range(NCORES)))
    LAST_RESULTS = res
    ys = [res.results[c]["y"].astype(np.float32) for c in range(NCORES)]
    npc = NCORES // B
    out = np.stack(
        [sum(ys[b * npc + 1 : (b + 1) * npc], ys[b * npc]) for b in range(B)]
    )
    return out.astype(np.float32)
